# revision 10
# baseline (speedup 1.0000x reference)
"""DMV inside algorithm (Eisner chart DP, logsumexp semiring) on Trainium2.

Strategy
--------
Pure data parallelism over the batch: 4096 sentences -> 8 cores x 512.
Per core: 2 sequential "runs" of 256 sentences, each laid out as
[128 SBUF partitions] x [G=2 sentence groups in the free dim].

The DP runs in the *exp domain* (no per-split transcendentals): tables hold
exp(score) and each width-w update is one fused strided multiply + one fused
segmented reduce on VectorE, plus a handful of small fixup ops.

Tables are stored *diagonal-packed*: Xd[d*41 + i] = X[i, i+d], which makes
every gather in the width-w recurrence a regular (constant-stride) access
pattern. IR/IL are stored with row r holding width r+1 (IL additionally
column-shifted by +1) so that all four quantities' gathers share one AP.

Numerics: scale composes linearly in span width (every width-w entry contains
exactly w arcs), so on-device renormalization at w=14 and w=28 multiplies
row d by exp(delta*d) (and the per-arc constant tables by exp(delta)),
keeping everything in f32 range. The accumulated delta is returned per
sentence and undone on the host: LL = log(CR[0,len]) - dsum*len.
"""

import os

os.environ.setdefault("JAX_PLATFORMS", "cpu")

import numpy as np

import concourse.bass as bass  # noqa: F401  (registers engine classes)
import concourse.tile as tile
import bass_rust
from concourse import bacc, mybir

F32 = mybir.dt.float32
AF = mybir.ActivationFunctionType
OP = mybir.AluOpType
AX = mybir.AxisListType

N = 41              # fake_len (ROOT at 0)
D = 1681            # table pitch: N*N elements
G = 2               # sentence groups per partition
RUNS = 2            # runs per core (2 * 128 * G = 512 sentences)
NCORES = 8
B_CORE = RUNS * 128 * G
CONST_IN = 4 * D          # host sends 4 pre-exponentiated tables/sentence
STOP_IN = 8 * N           # host sends 8 exp'd stop/go vectors/sentence
RENORM_AT = (20,)

# banks tile: 8 diag-packed tables x 2 groups; slots arranged in 4 blocks of
# 4 so each big op's (q,g) gather is ONE fused AP dim (ISA: max 3 free dims):
#   opA in0: [KR_g0 KR_g1 CRa_g0 CRa_g1]  slots 0..3
#   opA in1: [CLb_g0 CLb_g1 KL_g0 KL_g1]  slots 4..7
#   opB in0: [IR_g0 IR_g1 CLa_g0 CLa_g1]  slots 8..11
#   opB in1: [CRb_g0 CRb_g1 IL_g0 IL_g1]  slots 12..15 (IL stored col+1)
S_KR, S_CRA, S_CLB, S_KL, S_IR, S_CLA, S_CRB, S_IL = (
    0, 2, 4, 6, 8, 10, 12, 14)
# consts tile: 4 per-arc tables, g-major: offset (4*g + C)*D
C_A1, C_B1, C_DA, C_DB = range(4)
# stops tile: 16 vectors of 41, offset (g*8 + v)*41
V_GL0, V_GL1, V_GR0, V_GR1, V_SLNO, V_SLHAS, V_SRNO, V_SRHAS = range(8)

# scratch tile element offsets
Z_P = 0          # 1680: products [qg,i,t]
Z_SSUM = 1680    # 164
Z_TMP1 = 1844    # 164
Z_TMP2 = 2008    # 164
Z_M2 = 2172      # 4
Z_MU = 2176      # 2
Z_LM = 2178      # 2 (reused for delta)
Z_M = 2180       # 84: renorm multiplier table [g, 42]
Z_CROUT = 2264   # 82
Z_DSUM = 2346    # 2
Z_IOTA = 2348    # 42
Z_TOTAL = 2390

LN2_32 = 32.0 * float(np.log(2.0))


def ap_of(t, offset, dims, lead=None):
    """Build a raw AP on tile/dram ap `t`: [lead or t.ap[0]] + dims."""
    ap = t.copy()
    first = list(t.ap[0]) if lead is None else list(lead)
    ap.ap = bass_rust.VecI64Pair([first] + [list(d) for d in dims])
    ap.offset = offset
    return ap


def build_nc():
    nc = bacc.Bacc("TRN2", target_bir_lowering=False, debug=False, num_devices=1)
    consts_in = nc.dram_tensor("consts", [B_CORE, CONST_IN], F32, kind="ExternalInput").ap()
    stops_in = nc.dram_tensor("stops", [B_CORE, STOP_IN], F32, kind="ExternalInput").ap()
    iota_d = nc.dram_tensor("iota", [42], F32, kind="ExternalInput").ap()
    logs_d = nc.dram_tensor("ecr", [B_CORE, N], F32, kind="ExternalOutput").ap()
    dsum_d = nc.dram_tensor("dsum", [B_CORE], F32, kind="ExternalOutput").ap()

    with tile.TileContext(nc) as tc:
        with tc.tile_pool(name="p", bufs=1) as pool:
            banks_t = pool.tile([128, 16 * D], F32)
            consts_t = pool.tile([128, 8 * D], F32)
            stops_t = pool.tile([128, 16 * N], F32)
            z_t = pool.tile([128, Z_TOTAL], F32)
            banks = banks_t[:]
            consts = consts_t[:]
            stops = stops_t[:]
            z = z_t[:]

            v = nc.vector
            sc = nc.scalar

            # iota constant (once)
            nc.sync.dma_start(
                ap_of(z, Z_IOTA, [[1, 42]]),
                ap_of(iota_d, 0, [[1, 42]], lead=[0, 128]),
            )

            for r in range(RUNS):
                base_s = r * 256  # first sentence of this run (per core)

                # ---- load host-precomputed exp-domain constants ----
                nc.sync.dma_start(
                    ap_of(stops, 0, [[STOP_IN, G], [1, STOP_IN]]),
                    ap_of(stops_in, base_s * STOP_IN,
                          [[STOP_IN, G], [1, STOP_IN]], lead=[G * STOP_IN, 128]),
                )
                for g in range(G):
                    nc.sync.dma_start(
                        ap_of(consts, 4 * g * D, [[1, CONST_IN]]),
                        ap_of(consts_in, (base_s + g) * CONST_IN,
                              [[1, CONST_IN]], lead=[G * CONST_IN, 128]),
                    )

                # ---- width-0 init ----
                nc.gpsimd.memset(banks, 0.0)
                v.memset(ap_of(z, Z_DSUM, [[1, 2]]), 0.0)
                # KR[0,:] = KL[0,:] = 1
                v.memset(ap_of(banks, S_KR * D, [[6 * D, 2], [D, 2], [1, N]]), 1.0)
                # CRa/CRb[0,i] = exp(stop[i,R,NO])
                v.tensor_copy(
                    ap_of(banks, S_CRA * D, [[10 * D, 2], [D, 2], [1, N]]),
                    ap_of(stops, V_SRNO * N, [[0, 2], [8 * N, 2], [1, N]]),
                )
                # CLa/CLb[0,i] = exp(stop[i,L,NO])
                v.tensor_copy(
                    ap_of(banks, S_CLA * D, [[-6 * D, 2], [D, 2], [1, N]]),
                    ap_of(stops, V_SLNO * N, [[0, 2], [8 * N, 2], [1, N]]),
                )

                # ---- chart DP ----
                for w in range(1, N):
                    s = N - w
                    row = (w - 1) * N + 1
                    # opA: P[qg,i,t] = {KR,CRa}[t,i] * {CLb,KL}[w-1-t, i+t+1]
                    pa = ap_of(z, Z_P, [[s * w, 4], [w, s], [1, w]])
                    v.tensor_tensor(
                        pa,
                        ap_of(banks, S_KR * D, [[D, 4], [1, s], [N, w]]),
                        ap_of(banks, S_CLB * D + row, [[D, 4], [1, s], [-40, w]]),
                        OP.mult,
                    )
                    v.reduce_sum(
                        ap_of(z, Z_SSUM, [[s, 4], [1, s]]), pa, axis=AX.X,
                    )
                    # tmp1 = Ssum * {A1,B1}[w,:]
                    v.tensor_tensor(
                        ap_of(z, Z_TMP1, [[2 * s, 2], [s, 2], [1, s]]),
                        ap_of(z, Z_SSUM, [[2 * s, 2], [s, 2], [1, s]]),
                        ap_of(consts, C_A1 * D + w * N, [[D, 2], [4 * D, 2], [1, s]]),
                        OP.mult,
                    )
                    # tmp2 = {CLb[w-1,1+i], CRa[w-1,i]} * {DA,DB}[w,:]
                    # (independent of reduceA -> run on GpSimd in parallel)
                    nc.gpsimd.tensor_tensor(
                        ap_of(z, Z_TMP2, [[2 * s, 2], [s, 2], [1, s]]),
                        ap_of(banks, S_CLB * D + row, [[-2 * D - 1, 2], [D, 2], [1, s]]),
                        ap_of(consts, C_DA * D + w * N, [[D, 2], [4 * D, 2], [1, s]]),
                        OP.mult,
                    )
                    # IR[w-1, i] / IL[w-1, i+1] = tmp1 + tmp2
                    v.tensor_tensor(
                        ap_of(banks, S_IR * D + (w - 1) * N, [[6 * D + 1, 2], [D, 2], [1, s]]),
                        ap_of(z, Z_TMP1, [[2 * s, 2], [s, 2], [1, s]]),
                        ap_of(z, Z_TMP2, [[2 * s, 2], [s, 2], [1, s]]),
                        OP.add,
                    )
                    # opB: P[qg,i,t] = {IR,CLa}[t,i] * {CRb,IL}[w-1-t, i+t+1]
                    v.tensor_tensor(
                        pa,
                        ap_of(banks, S_IR * D, [[D, 4], [1, s], [N, w]]),
                        ap_of(banks, S_CRB * D + row, [[D, 4], [1, s], [-40, w]]),
                        OP.mult,
                    )
                    # KR[w,i], KL[w,i] = sum_t
                    v.reduce_sum(
                        ap_of(banks, S_KR * D + w * N, [[6 * D, 2], [D, 2], [1, s]]),
                        pa, axis=AX.X,
                    )
                    # CRa/CRb[w,i] = KR[w,i] * sRhas[i]
                    v.tensor_tensor(
                        ap_of(banks, S_CRA * D + w * N, [[10 * D, 2], [D, 2], [1, s]]),
                        ap_of(banks, S_KR * D + w * N, [[0, 2], [D, 2], [1, s]]),
                        ap_of(stops, V_SRHAS * N, [[0, 2], [8 * N, 2], [1, s]]),
                        OP.mult,
                    )
                    # CLa/CLb[w,i] = KL[w,i] * sLhas[i+w]
                    v.tensor_tensor(
                        ap_of(banks, S_CLA * D + w * N, [[-6 * D, 2], [D, 2], [1, s]]),
                        ap_of(banks, S_KL * D + w * N, [[0, 2], [D, 2], [1, s]]),
                        ap_of(stops, V_SLHAS * N + w, [[0, 2], [8 * N, 2], [1, s]]),
                        OP.mult,
                    )

                    if w in RENORM_AT:
                        s0 = N - w
                        # mu[g] = max_i max(KR[w,i], KL[w,i])
                        v.reduce_max(
                            ap_of(z, Z_M2, [[2, 2], [1, 2]]),
                            ap_of(banks, S_KR * D + w * N, [[6 * D, 2], [D, 2], [1, s0]]),
                            axis=AX.X,
                        )
                        v.tensor_tensor(
                            ap_of(z, Z_MU, [[1, 2]]),
                            ap_of(z, Z_M2, [[1, 2]]),
                            ap_of(z, Z_M2 + 2, [[1, 2]]),
                            OP.max,
                        )
                        # Ln range on ACT is +-2^64: compute via mu*2^-32
                        v.tensor_scalar_mul(
                            ap_of(z, Z_MU, [[1, 2]]), ap_of(z, Z_MU, [[1, 2]]), 2.0**-32
                        )
                        v.tensor_scalar_max(
                            ap_of(z, Z_MU, [[1, 2]]), ap_of(z, Z_MU, [[1, 2]]), 1e-36
                        )
                        sc.activation(
                            ap_of(z, Z_LM, [[1, 2]]), ap_of(z, Z_MU, [[1, 2]]), AF.Ln
                        )
                        # quantize the per-width shift to delta = -k*ln2 with
                        # k integer, so every rescale factor is an EXACT power
                        # of two (the ACT exp LUT would otherwise perturb all
                        # tables by its relative error).
                        # kf = round((log(mu*2^-32) + 32 ln2) / (w ln2))
                        v.tensor_scalar(
                            ap_of(z, Z_LM, [[1, 2]]), ap_of(z, Z_LM, [[1, 2]]),
                            LN2_32, 1.0 / (w * float(np.log(2.0))),
                            OP.add, OP.mult,
                        )
                        v.tensor_scalar(
                            ap_of(z, Z_LM, [[1, 2]]), ap_of(z, Z_LM, [[1, 2]]),
                            12582912.0, 12582912.0, OP.add, OP.subtract,
                        )
                        # dsum accumulates k (exact small integers)
                        v.tensor_tensor(
                            ap_of(z, Z_DSUM, [[1, 2]]),
                            ap_of(z, Z_DSUM, [[1, 2]]),
                            ap_of(z, Z_LM, [[1, 2]]),
                            OP.add,
                        )
                        # scale2 = 2^-k via exponent bits: (127 - k) << 23
                        v.tensor_scalar(
                            ap_of(z, Z_M2, [[1, 2]]), ap_of(z, Z_LM, [[1, 2]]),
                            -1.0, 127.0, OP.mult, OP.add,
                        )
                        zi = z.bitcast(mybir.dt.int32)
                        v.tensor_copy(
                            ap_of(zi, Z_M2 + 2, [[1, 2]]),
                            ap_of(z, Z_M2, [[1, 2]]),
                        )
                        v.tensor_scalar(
                            ap_of(zi, Z_M2 + 2, [[1, 2]]),
                            ap_of(zi, Z_M2 + 2, [[1, 2]]),
                            23, None, OP.arith_shift_left,
                        )
                        # M[g, d] = 2^(-k*d): d=0 -> 1, then multiplicative scan
                        v.memset(ap_of(z, Z_M, [[42, 2], [1, 1]]), 1.0)
                        for g in range(G):
                            sca = ap_of(z, Z_M2 + 2 + g, [[0, 41]])
                            v.tensor_tensor_scan(
                                ap_of(z, Z_M + g * 42 + 1, [[1, 41]]),
                                sca, sca, 1.0, OP.mult, OP.bypass,
                            )
                        for g in range(G):
                            # natural tables, rows d<=w: scale by exp(delta*d)
                            tA = ap_of(banks, g * D, [[2 * D, 4], [N, w + 1], [1, N]])
                            v.tensor_tensor(
                                tA, tA,
                                ap_of(z, Z_M + g * 42, [[0, 4], [1, w + 1], [0, N]]),
                                OP.mult,
                            )
                            tB = ap_of(banks, (10 + g) * D, [[2 * D, 2], [N, w + 1], [1, N]])
                            v.tensor_tensor(
                                tB, tB,
                                ap_of(z, Z_M + g * 42, [[0, 2], [1, w + 1], [0, N]]),
                                OP.mult,
                            )
                            # IR/IL rows r<=w-1 hold width r+1: exp(delta*(r+1))
                            tI = ap_of(banks, (8 + g) * D, [[6 * D, 2], [N, w], [1, N]])
                            v.tensor_tensor(
                                tI, tI,
                                ap_of(z, Z_M + g * 42 + 1, [[0, 2], [1, w], [0, N]]),
                                OP.mult,
                            )
                            # const rows > w: one extra arc factor exp(delta)
                            tC = ap_of(consts, 4 * g * D + (w + 1) * N,
                                       [[D, 4], [N, 40 - w], [1, N]])
                            v.tensor_tensor(
                                tC, tC,
                                ap_of(z, Z_M + g * 42 + 1, [[0, 4], [0, 40 - w], [0, N]]),
                                OP.mult,
                            )

                # ---- extract raw exp-domain CR[0, j] (log on host) ----
                v.tensor_copy(
                    ap_of(z, Z_CROUT, [[N, 2], [1, N]]),
                    ap_of(banks, S_CRA * D, [[D, 2], [N, N]]),
                )
                nc.sync.dma_start(
                    ap_of(logs_d, base_s * N, [[N, G], [1, N]], lead=[G * N, 128]),
                    ap_of(z, Z_CROUT, [[N, G], [1, N]]),
                )
                nc.sync.dma_start(
                    ap_of(dsum_d, base_s, [[1, G]], lead=[G, 128]),
                    ap_of(z, Z_DSUM, [[1, G]]),
                )

    nc.compile()
    return nc


_NC_CACHE = {}


def get_nc():
    if "nc" not in _NC_CACHE:
        _NC_CACHE["nc"] = build_nc()
    return _NC_CACHE["nc"]


def make_in_maps(trans_scores, dec_scores):
    t = np.asarray(trans_scores, dtype=np.float32)
    dec = np.asarray(dec_scores, dtype=np.float32)
    B = t.shape[0]
    go = dec[..., 0]                        # [B, n, dir, dv]
    # per-sentence linear pre-shift: each arc factor carries exp(-c0), so a
    # width-w entry is scaled exp(-c0*w); undone on the host at the end.
    tm = np.where(t < -1e8, -np.inf, t)
    with np.errstate(invalid="ignore"):
        proxy = np.nanmean(np.where(np.isfinite(tm.max(axis=(1, 3))), tm.max(axis=(1, 3)), np.nan)[:, 1:], axis=-1)
    c0 = (proxy + 0.5).astype(np.float32)
    c0 = np.clip(np.nan_to_num(c0), -20.0, 20.0)
    d_idx, i_idx = np.meshgrid(np.arange(N), np.arange(N), indexing="ij")
    j_idx = np.minimum(i_idx + d_idx, N - 1)
    valid = ((i_idx + d_idx) <= N - 1).astype(np.float32)
    cc = c0[:, None, None]
    # per-arc exp tables, diag-packed [d, i]
    a1 = np.exp(t[:, i_idx, j_idx, 1] + go[:, :, 1, 1][:, i_idx] - cc) * valid
    a0 = np.exp(t[:, i_idx, j_idx, 0] + go[:, :, 1, 0][:, i_idx] - cc) * valid
    b1 = np.exp(t[:, j_idx, i_idx, 1] + go[:, :, 0, 1][:, j_idx] - cc) * valid
    b0 = np.exp(t[:, j_idx, i_idx, 0] + go[:, :, 0, 0][:, j_idx] - cc) * valid
    consts = np.stack([a1, b1, a0 - a1, b0 - b1], axis=1)  # [B, 4, n, n]
    consts = np.ascontiguousarray(consts.reshape(B, CONST_IN))
    st = dec[..., 1]                        # [B, n, dir, dv]
    stops = np.exp(np.stack([
        go[:, :, 0, 0], go[:, :, 0, 1], go[:, :, 1, 0], go[:, :, 1, 1],
        st[:, :, 0, 0], st[:, :, 0, 1], st[:, :, 1, 0], st[:, :, 1, 1],
    ], axis=1))                             # [B, 8, n]
    stops = np.ascontiguousarray(stops.reshape(B, STOP_IN).astype(np.float32))
    iota = np.arange(42, dtype=np.float32)
    in_maps = []
    for c in range(NCORES):
        sl = slice(c * B_CORE, (c + 1) * B_CORE)
        in_maps.append({
            "consts": consts[sl],
            "stops": stops[sl],
            "iota": iota,
        })
    return in_maps, c0


def assemble(results, len_array, c0):
    ln = np.asarray(len_array).astype(np.int64)
    c0 = np.asarray(c0).astype(np.float64)
    out = np.empty(len(ln), dtype=np.float32)
    for c, res in enumerate(results):
        ecr = res["ecr"].reshape(B_CORE, N).astype(np.float64)
        dsum = res["dsum"].reshape(B_CORE).astype(np.float64)
        lc = ln[c * B_CORE:(c + 1) * B_CORE]
        idx = np.arange(B_CORE)
        with np.errstate(divide="ignore"):
            out[c * B_CORE:(c + 1) * B_CORE] = (
                np.log(ecr[idx, lc]) + dsum * np.log(2.0) * lc
                + c0[c * B_CORE:(c + 1) * B_CORE] * lc
            ).astype(np.float32)
    return out


def kernel(trans_scores, dec_scores, len_array):
    from concourse.bass_utils import run_bass_kernel_spmd

    nc = get_nc()
    in_maps, c0 = make_in_maps(trans_scores, dec_scores)
    res = run_bass_kernel_spmd(nc, in_maps, core_ids=list(range(NCORES)))
    return assemble(res.results, len_array, c0)
